# revision 22
# baseline (speedup 1.0000x reference)
"""Trainium2 Bass kernel for nn_Attention (T=2048, D=2048, H=16, Dh=128).

Tensor-parallel over heads, 2 heads per core on 8 cores. fp8e4m3
DoubleRow matmuls (contraction pairs in one instruction) for the qkv
projections, PV, softmax-sum, and sum(x^2); scores and the output
projection stay bf16. Queries 0..127 (few softmax keys, no error
averaging) run on a small full-bf16 island. Weight scales (x32 wq/wk,
x16 wv) keep fp8 weights normal-range; exp gets a -5 softmax-invariant
bias so e fits e4m3's 240 max.

Schedule notes: the PE clock is HAM-gated (1.2 GHz cold, 2.4 GHz after
~3.4us of sustained busy), so the emission order packs PE work densely
from the first DMA onward and software-pipelines attention (scores of
pair p+1 issue before PV/sum of pair p) to keep the in-order PE queue
from stalling on the exp chain. The four per-strip sum(x^2) accumulators
are spread across ppv/psc/psums banks (DR dst must be partition-base 0)
so the pmm ring stays free for early matmul groups.
"""

import math
import os
import sys
import time

for _p in ("/opt/trn_rl_repo", "/root/.axon_site/_ro/trn_rl_repo"):
    if os.path.isdir(_p) and _p not in sys.path:
        sys.path.insert(0, _p)

import numpy as np
import ml_dtypes

import concourse.bass as bass
import concourse.tile as tile
from concourse import bacc, mybir
from concourse.bass_utils import run_bass_kernel_spmd

BF16 = mybir.dt.bfloat16
FP8 = mybir.dt.float8e4
F32R = mybir.dt.float32r
F32 = mybir.dt.float32
AF = mybir.ActivationFunctionType
DR = mybir.MatmulPerfMode.DoubleRow
E4NP = ml_dtypes.float8_e4m3

T = 2048
D = 2048
N_H = 16
D_H = 128
N_CORES = 8
H_LOC = N_H // N_CORES          # heads per core = 2
NL = H_LOC * D_H                # local head width = 256
KD = D // 128                   # contraction tiles = 16
KP = KD // 2                    # contraction tile pairs = 8
TT = T // 128                   # t tiles = 16
NS = T // 512                   # 512-wide strips = 4
EPS = 1e-5
WS = 32.0                       # wq/wk fp8 scale
WVS = 16.0                      # wv fp8 scale
EB = -5.0                       # exp bias (softmax-invariant shift)
SC_EXP = 1.0 / (WS * WS * math.sqrt(D_H))

_CACHED = {}


def _build_program(repeats=1):
    if repeats in _CACHED:
        return _CACHED[repeats]

    nc = bacc.Bacc("TRN2", target_bir_lowering=False, debug=False, num_devices=N_CORES)

    # all big inputs are host-pre-swizzled to [128, ...] partition-major so
    # every DMA is one contiguous segment per partition (cheap descriptors)
    x8_d = nc.dram_tensor("x8S", [128, NS, KD, 512], FP8, kind="ExternalInput")
    x16_d = nc.dram_tensor("x16S", [128, KD, 128], BF16, kind="ExternalInput")
    wq8_d = nc.dram_tensor("wq8S", [128, KD, NL], FP8, kind="ExternalInput")
    wk8_d = nc.dram_tensor("wk8S", [128, KD, NL], FP8, kind="ExternalInput")
    wv8_d = nc.dram_tensor("wv8S", [128, KD, NL], FP8, kind="ExternalInput")
    wq16_d = nc.dram_tensor("wq16S", [128, KD, NL], BF16, kind="ExternalInput")
    wk16_d = nc.dram_tensor("wk16S", [128, KD, NL], BF16, kind="ExternalInput")
    wv16_d = nc.dram_tensor("wv16S", [128, KD, NL], BF16, kind="ExternalInput")
    wo_d = nc.dram_tensor("woS", [128, H_LOC, T], BF16, kind="ExternalInput")
    cos_d = nc.dram_tensor("cosT", [D_H, T], BF16, kind="ExternalInput")
    sin_d = nc.dram_tensor("sinT", [D_H, T], BF16, kind="ExternalInput")
    msk_d = nc.dram_tensor("masks", [128, 128], BF16, kind="ExternalInput")
    onc_d = nc.dram_tensor("ones_col", [1, 128], F32R, kind="ExternalInput")
    on128_d = nc.dram_tensor("ones128", [128, 16], BF16, kind="ExternalInput")
    on82_d = nc.dram_tensor("ones82", [128, 2, 16], FP8, kind="ExternalInput")
    # output in [128, TT, D] tile-major layout; host un-swizzles
    out_d = nc.dram_tensor("out", [128, TT, D], BF16, kind="ExternalOutput")

    ap = lambda h: h.ap()
    out_ap = ap(out_d)

    from contextlib import ExitStack

    with tile.TileContext(nc) as tc, ExitStack() as ctx:
        P = ctx.enter_context  # noqa

        singles = P(tc.tile_pool(name="singles", bufs=1))
        scr = P(tc.tile_pool(name="scr", bufs=2))          # [128,KD,512] fp8 x^2
        rope = P(tc.tile_pool(name="rope", bufs=4))        # [128,512] bf16
        epool = P(tc.tile_pool(name="epool", bufs=6))      # [128,2,512] fp8 exp
        rbsp = P(tc.tile_pool(name="rbsp", bufs=2))        # [128,512] bf16 pv evac
        small = P(tc.tile_pool(name="small", bufs=2))      # [1,512] f32
        stage = P(tc.tile_pool(name="stage", bufs=4))      # [128,T] bf16 out
        pmm = P(tc.tile_pool(name="pmm", bufs=3, space="PSUM"))
        psc = P(tc.tile_pool(name="psc", bufs=3, space="PSUM"))
        ppv = P(tc.tile_pool(name="ppv", bufs=1, space="PSUM"))
        psums = P(tc.tile_pool(name="psums", bufs=1, space="PSUM"))

        def emit_body():
            # ---------------- phase 0: critical loads ------------------------------
            onc = singles.tile([1, 128], F32R, tag="onc")
            nc.gpsimd.dma_start(out=onc, in_=ap(onc_d))
            on128 = singles.tile([128, 16], BF16, tag="on128")
            nc.gpsimd.dma_start(out=on128, in_=ap(on128_d))
            on82 = singles.tile([128, 2, 16], FP8, tag="on82")
            nc.gpsimd.dma_start(out=on82, in_=ap(on82_d))

            xt8 = singles.tile([128, NS, KD, 512], FP8, tag="xt8")
            for c in range(4):
                cseg = slice(4 * c, 4 * c + 4)
                nc.sync.dma_start(out=xt8[:, 0, cseg], in_=ap(x8_d)[:, 0, cseg])

            def load_w(eng, dram, n, dt, tag):
                t_ = singles.tile([128, KD, n], dt, tag=tag)
                eng.dma_start(out=t_, in_=ap(dram))
                return t_

            cosR = singles.tile([128, T], BF16, tag="cosR")
            nc.scalar.dma_start(out=cosR, in_=ap(cos_d))
            sinR = singles.tile([128, T], BF16, tag="sinR")
            nc.scalar.dma_start(out=sinR, in_=ap(sin_d))
            wq8 = load_w(nc.sync, wq8_d, NL, FP8, "wq8")
            wk8 = load_w(nc.sync, wk8_d, NL, FP8, "wk8")
            wv8 = singles.tile([128, KD, NL], FP8, tag="wv8")
            nc.sync.dma_start(out=wv8, in_=ap(wv8_d))
            epsb = singles.tile([1, 1], F32, tag="epsb")
            nc.vector.memset(epsb, EPS)
            ebias = singles.tile([128, 1], F32, tag="ebias")
            nc.vector.memset(ebias, EB)
            idt = singles.tile([1, 1], F32, tag="idt")
            nc.vector.memset(idt, 1.0)

            # ---------------- phase 1: sum(x^2) via fp8 squares + DR ones-matmul ---
            # DR ones-weights need a 16-elem k-pair stride, so the ones are
            # [128, 2, 16] and sums land in 16 identical rows; row 0 is read.
            ssq_dst = {}

            def emit_ssq_strip(j):
                # DR dst must sit at partition base 0; spread the four strip
                # sums across pools that are idle until their phase-3 readers
                if j == 0:
                    ssq_dst[j] = ppv.tile([16, 512], F32, tag="pv", name="ssq0")
                elif j == 3:
                    ssq_dst[j] = psums.tile([16, 512], F32, tag="su", name="ssq3")
                else:
                    ssq_dst[j] = psc.tile([16, 512], F32, tag="sc", name=f"ssq{j}")
                xsq = scr.tile([128, KD, 512], FP8, tag="xsq")
                for c in range(4):
                    cseg = slice(4 * c, 4 * c + 4)
                    if c % 2 == 0:
                        nc.scalar.activation(xsq[:, cseg, :], xt8[:, j, cseg, :], AF.Square)
                    else:
                        nc.vector.tensor_mul(xsq[:, cseg, :], xt8[:, j, cseg, :], xt8[:, j, cseg, :])
                for p in range(KP):
                    nc.tensor.matmul(
                        ssq_dst[j], lhsT=on82, rhs=xsq[:, 2 * p : 2 * p + 2, :],
                        start=(p == 0), stop=(p == KP - 1), perf_mode=DR,
                    )

            emit_ssq_strip(0)

            # ---------------- phase 2: remaining loads -----------------------------
            wv16 = singles.tile([128, KD, NL], BF16, tag="wv16")
            wo = singles.tile([128, H_LOC, T], BF16, tag="wo")
            msk = singles.tile([128, 128], BF16, tag="msk")
            x16 = singles.tile([128, KD, 128], BF16, tag="x16")
            wq16 = singles.tile([128, KD, NL], BF16, tag="wq16")
            wk16 = singles.tile([128, KD, NL], BF16, tag="wk16")

            def emit_loads2():
                nc.sync.dma_start(out=msk, in_=ap(msk_d))
                nc.scalar.dma_start(out=x16, in_=ap(x16_d))
                nc.scalar.dma_start(out=wq16, in_=ap(wq16_d))
                nc.scalar.dma_start(out=wk16, in_=ap(wk16_d))
                nc.scalar.dma_start(out=wv16, in_=ap(wv16_d))
                nc.gpsimd.dma_start(out=wo, in_=ap(wo_d))

            # ---------------- phase 3+4: per-strip s = exp(-0.5 ln(mean+eps)) ------
            s_m = singles.tile([1, T], F32, tag="s_m")
            s_row = singles.tile([1, T], F32R, tag="srow")
            cos_s = singles.tile([128, T], BF16, tag="cos_s")
            sin_s = singles.tile([128, T], BF16, tag="sin_s")
            sk_t = singles.tile([128, TT], F32, tag="sk")

            def emit_s_strip(j):
                js = slice(j * 512, (j + 1) * 512)
                nc.vector.tensor_copy(s_m[:, js], ssq_dst[j][0:1, :])
                nc.scalar.activation(s_m[:, js], s_m[:, js], AF.Ln, bias=epsb, scale=1.0 / D)
                nc.scalar.activation(s_row[:, js], s_m[:, js], AF.Exp, scale=-0.5)
                sb = pmm.tile([128, 512], F32, tag="mm")
                nc.tensor.matmul(sb, lhsT=onc, rhs=s_row[:, js], start=True, stop=True)
                nc.vector.tensor_mul(cos_s[:, js], cosR[:, js], sb)
                nc.vector.tensor_mul(sin_s[:, js], sinR[:, js], sb)
                # s into [128, TT] tile layout for the v scale: 4 tiny PE
                # transposes ([1,128] -> [128,1]) instead of a DRAM round trip
                sbt = pmm.tile([128, 512], F32, tag="mm")
                srf = s_row.bitcast(F32)
                for c in range(4):
                    nc.tensor.matmul(
                        sbt[:, c : c + 1],
                        lhsT=srf[:, j * 512 + 128 * c : j * 512 + 128 * (c + 1)],
                        rhs=idt, is_transpose=True, start=True, stop=True,
                    )
                nc.vector.tensor_copy(sk_t[:, 4 * j : 4 * j + 4], sbt[:, 0:4])

            # ---------------- phase 5: tiles + emit helpers ------------------------
            q_sb = singles.tile([128, H_LOC, T], BF16, tag="q_sb")
            k_sb = singles.tile([128, H_LOC, T], BF16, tag="k_sb")
            v8_sb = singles.tile([128, TT, NL], FP8, tag="v8_sb")
            v16_0 = singles.tile([128, NL], BF16, tag="v16_0")
            q16i = singles.tile([128, H_LOC, 128], BF16, tag="q16i")
            k16i = singles.tile([128, H_LOC, 128], BF16, tag="k16i")
            qr16i = singles.tile([128, H_LOC, 128], BF16, tag="qr16i")
            kr16i = singles.tile([128, H_LOC, 128], BF16, tag="kr16i")
            qw16i = singles.tile([128, H_LOC, 128], BF16, tag="qw16i")
            kw16i = singles.tile([128, H_LOC, 128], BF16, tag="kw16i")
            outT = singles.tile([128, H_LOC, T], BF16, tag="outT")

            def emit_rope(dst_ap, ps, js, n):
                # m1 = q*cos_s; m2 = swap(q)*sin_s (rotate_half sign folded
                # into sin table rows 0-63 on host)
                m1 = rope.tile([128, 512], BF16, tag="m1")
                nc.vector.tensor_mul(m1[:, :n], ps[:, :n], cos_s[:, js])
                m2 = rope.tile([128, 512], BF16, tag="m2")
                nc.vector.tensor_mul(m2[0:64, :n], ps[64:128, :n], sin_s[0:64, js])
                nc.vector.tensor_mul(m2[64:128, :n], ps[0:64, :n], sin_s[64:128, js])
                nc.vector.tensor_add(dst_ap, m1[:, :n], m2[:, :n])

            def emit_qk_strip(h, j):
                hs = slice(h * 128, (h + 1) * 128)
                js = slice(j * 512, (j + 1) * 512)
                for dst, w in ((k_sb, wk8), (q_sb, wq8)):
                    ps = pmm.tile([128, 512], F32, tag="mm")
                    for p in range(KP):
                        nc.tensor.matmul(
                            ps, lhsT=w[:, 2 * p : 2 * p + 2, hs],
                            rhs=xt8[:, j, 2 * p : 2 * p + 2, :],
                            start=(p == 0), stop=(p == KP - 1), perf_mode=DR,
                        )
                    emit_rope(dst[:, h, js], ps, js, 512)

            def emit_island_qk_mm():
                # evacuate raw + partition-swapped copies so the deferred rope
                # reads SBUF operands at matching base partitions
                for raw, sw, w in ((kr16i, kw16i, wk16), (qr16i, qw16i, wq16)):
                    for h in range(H_LOC):
                        hs = slice(h * 128, (h + 1) * 128)
                        ps = pmm.tile([128, 512], F32, tag="mm")
                        for kd in range(KD):
                            nc.tensor.matmul(
                                ps[:, 0:128], lhsT=w[:, kd, hs], rhs=x16[:, kd, :],
                                start=(kd == 0), stop=(kd == KD - 1),
                            )
                        nc.scalar.copy(raw[:, h, :], ps[:, 0:128])
                        nc.scalar.copy(sw[0:64, h, :], ps[64:128, 0:128])
                        nc.scalar.copy(sw[64:128, h, :], ps[0:64, 0:128])

            def emit_island_rope():
                for dst, raw, sw in ((k16i, kr16i, kw16i), (q16i, qr16i, qw16i)):
                    for h in range(H_LOC):
                        m1 = rope.tile([128, 512], BF16, tag="m1")
                        nc.vector.tensor_mul(m1[:, 0:128], raw[:, h, :], cos_s[:, 0:128])
                        m2 = rope.tile([128, 512], BF16, tag="m2")
                        nc.vector.tensor_mul(m2[:, 0:128], sw[:, h, :], sin_s[:, 0:128])
                        nc.vector.tensor_add(dst[:, h, :], m1[:, 0:128], m2[:, 0:128])

            def emit_island_v():
                ps = pmm.tile([128, 512], F32, tag="mm")
                for kd in range(KD):
                    nc.tensor.matmul(
                        ps[:, 0:NL], lhsT=x16[:, kd, :], rhs=wv16[:, kd, :],
                        start=(kd == 0), stop=(kd == KD - 1),
                    )
                nc.vector.tensor_scalar_mul(v16_0, ps[:, 0:NL], sk_t[:, 0:1])

            def emit_v_tile(tt):
                ts = slice((tt % 4) * 128, (tt % 4 + 1) * 128)
                ps = pmm.tile([128, 512], F32, tag="mm")
                for p in range(KP):
                    nc.tensor.matmul(
                        ps[:, 0:NL], lhsT=xt8[:, tt // 4, 2 * p : 2 * p + 2, ts],
                        rhs=wv8[:, 2 * p : 2 * p + 2, :],
                        start=(p == 0), stop=(p == KP - 1), perf_mode=DR,
                    )
                nc.vector.tensor_scalar_mul(
                    v8_sb[:, tt, :], ps[:, 0:NL], sk_t[:, tt : tt + 1]
                )

            def emit_attention(h, j):
                hs = slice(h * 128, (h + 1) * 128)
                js = slice(j * 512, (j + 1) * 512)
                po = ppv.tile([128, 512], F32, tag="pv")
                su = psums.tile([16, 512], F32, tag="su")

                # pair descriptors: (a, b, c0p, start_flag)
                if j == 0:
                    pairs = [(0, 1, 128, True), (2, 3, 256, False)]
                else:
                    pairs = [(2 * p, 2 * p + 1, 0, p == 0) for p in range(2 * j)] + \
                            [(4 * j, 4 * j + 1, 0, False),
                             (4 * j + 2, 4 * j + 3, 256, False)]

                def emit_scores_exp(pair):
                    a, b, c0p, _ = pair
                    e8 = epool.tile([128, 2, 512], FP8, tag="e")
                    for slot, ti in ((0, a), (1, b)):
                        r = ti - 4 * j
                        c0s = max(c0p, 128 * r if r >= 0 else 0)
                        island = (j == 0 and ti == 0)
                        if island:
                            c0s = 128
                        st = psc.tile([128, 512], F32, tag="sc")
                        qs = slice(j * 512 + c0s, (j + 1) * 512)
                        nc.tensor.matmul(
                            st[:, c0s:512],
                            lhsT=k_sb[:, h, ti * 128 : (ti + 1) * 128],
                            rhs=q_sb[:, h, qs], start=True, stop=True,
                        )
                        nc.scalar.activation(e8[:, slot, c0s:512], st[:, c0s:512],
                                             AF.Exp, bias=ebias, scale=SC_EXP)
                        if r >= 0 and not island:
                            nc.vector.tensor_mul(
                                e8[:, slot, c0s : c0s + 128],
                                e8[:, slot, c0s : c0s + 128], msk,
                            )
                        if c0s > c0p:
                            nc.gpsimd.memset(e8[:, slot, c0p:c0s], 0.0)
                    return e8

                def emit_pv_su(pair, e8, last):
                    a, b, c0p, st_flag = pair
                    cs = slice(c0p, 512)
                    nc.tensor.matmul(
                        po[:, cs], lhsT=v8_sb[:, a : a + 2, hs], rhs=e8[:, :, cs],
                        start=st_flag, stop=last, perf_mode=DR,
                    )
                    nc.tensor.matmul(
                        su[:, cs], lhsT=on82, rhs=e8[:, :, cs],
                        start=st_flag, stop=last, perf_mode=DR,
                    )

                if j == 0:
                    # island: queries 0..127 x keys 0..127 fully bf16
                    sti = psc.tile([128, 512], F32, tag="sc")
                    nc.tensor.matmul(
                        sti[:, 0:128], lhsT=k16i[:, h, :], rhs=q16i[:, h, :],
                        start=True, stop=True,
                    )
                    e16 = epool.tile([128, 128], BF16, tag="e16")
                    nc.scalar.activation(e16, sti[:, 0:128], AF.Exp,
                                         bias=ebias, scale=SC_EXP)
                    nc.vector.tensor_mul(e16, e16, msk)
                    nc.tensor.matmul(po[:, 0:128], lhsT=v16_0[:, hs], rhs=e16,
                                     start=True, stop=False)
                    nc.tensor.matmul(su[:, 0:128], lhsT=on128, rhs=e16,
                                     start=True, stop=False)

                # software pipeline: scores/exp of pair p+1 issue before pv/su
                # of pair p so the in-order PE queue never waits on the exp;
                e8_prev = emit_scores_exp(pairs[0])
                for pi in range(1, len(pairs)):
                    e8_next = emit_scores_exp(pairs[pi])
                    emit_pv_su(pairs[pi - 1], e8_prev, last=False)
                    e8_prev = e8_next
                emit_pv_su(pairs[-1], e8_prev, last=True)

                pos = rbsp.tile([128, 512], BF16, tag="pos")
                nc.scalar.copy(pos, po)
                lnr = small.tile([1, 512], F32, tag="lnr")
                nc.scalar.activation(lnr, su[0:1, :], AF.Ln)
                rec = small.tile([1, 512], F32R, tag="rec")
                nc.scalar.activation(rec, lnr, AF.Exp, scale=-1.0)
                rb = psums.tile([128, 512], F32, tag="su")
                nc.tensor.matmul(rb, lhsT=onc, rhs=rec, start=True, stop=True)
                nc.vector.tensor_mul(outT[:, h, js], rb, pos)

            def emit_wo_tile(tt):
                ts = slice(tt * 128, (tt + 1) * 128)
                stg = stage.tile([128, T], BF16, tag="stg")
                for n in range(NS):
                    ns = slice(n * 512, (n + 1) * 512)
                    ps = pmm.tile([128, 512], F32, tag="mm")
                    for h in range(H_LOC):
                        nc.tensor.matmul(
                            ps, lhsT=outT[:, h, ts], rhs=wo[:, h, ns],
                            start=(h == 0), stop=(h == H_LOC - 1),
                        )
                    if n % 2 == 0:
                        nc.vector.tensor_copy(stg[:, ns], ps)
                    else:
                        nc.scalar.copy(stg[:, ns], ps)
                nc.gpsimd.dma_start(out=out_ap[:, tt, :], in_=stg)

            # ---------------- emission sequence ------------------------------------
            # priority: dense PE work from the first DMA onward (HAM warmup).
            # The strip-0 chain (ssq0 -> s0 -> rope tables -> qk/v strip 0 ->
            # attention) is completed first; later strips' ssq/s work is
            # interleaved where its x8 DMA will have landed, so the in-order
            # PE queue never has a far-future wait at its head.
            # x8 strips: s1 on scalar (after cos/sin), s2/s3 on sync
            nc.scalar.dma_start(out=xt8[:, 1], in_=ap(x8_d)[:, 1])
            nc.sync.dma_start(out=xt8[:, 2], in_=ap(x8_d)[:, 2])
            nc.sync.dma_start(out=xt8[:, 3], in_=ap(x8_d)[:, 3])
            emit_loads2()
            emit_s_strip(0)
            emit_ssq_strip(1)
            emit_s_strip(1)
            emit_qk_strip(0, 0)
            emit_qk_strip(1, 0)
            emit_qk_strip(0, 1)
            emit_qk_strip(1, 1)
            for tt in range(8):
                emit_v_tile(tt)
            # strip 1 first: its attention needs no island, which buys time
            # for the island's bf16 weights to land on the scalar queue
            emit_attention(0, 1)
            emit_attention(1, 1)
            emit_island_qk_mm()
            emit_island_rope()
            emit_island_v()
            emit_ssq_strip(2)
            emit_s_strip(2)
            emit_ssq_strip(3)
            emit_s_strip(3)
            for tt in range(4, 8):
                emit_wo_tile(tt)
            emit_attention(0, 0)
            emit_attention(1, 0)
            emit_qk_strip(0, 2)
            emit_qk_strip(1, 2)
            emit_qk_strip(0, 3)
            emit_qk_strip(1, 3)
            for tt in range(8, 16):
                emit_v_tile(tt)
            for tt in range(0, 4):
                emit_wo_tile(tt)
            # strip 3 before strip 2: the final (serial-tail) strip is shorter
            emit_attention(0, 3)
            emit_attention(1, 3)
            for tt in range(12, 16):
                emit_wo_tile(tt)
            emit_attention(0, 2)
            emit_attention(1, 2)
            for tt in range(8, 12):
                emit_wo_tile(tt)

        for _rep in range(repeats):
            emit_body()

    # Force Exp and Ln onto the single combined table set: drop them from
    # every other set in the (cached, order-preserving) table map so the
    # table-load pass picks natural_log_exp_and_others for both — one
    # ACT_TABLE_LOAD for the whole kernel instead of per-strip thrash.
    from concourse.hw_specs import get_activation_tables
    tabs = get_activation_tables(nc.m.arch)
    for nm_, fs_ in tabs.items():
        if nm_ != "natural_log_exp_and_others":
            fs_.discard(AF.Exp)
            fs_.discard(AF.Ln)
    nc.compile()
    _CACHED[repeats] = nc
    return nc


def _host_prep(x, w_ln, wq, wk, wv, wo, cos, sin):
    bf = ml_dtypes.bfloat16
    x = np.asarray(x, np.float32)
    w_ln = np.asarray(w_ln, np.float32)
    cosT = np.ascontiguousarray(np.asarray(cos, np.float32).T).astype(bf)
    sinTf = np.ascontiguousarray(np.asarray(sin, np.float32).T)
    sinTf[0:64] *= -1.0          # rotate_half sign folded into the table
    sinT = sinTf.astype(bf)
    xT = np.ascontiguousarray(x.T)

    def swz(a):
        # [D_, N] -> [128, D_//128, N] partition-major
        d_, n_ = a.shape
        return np.ascontiguousarray(a.reshape(d_ // 128, 128, n_).transpose(1, 0, 2))

    # x8S[p, j, n, c] = xT[n*128+p, j*512+c]
    x8S = np.ascontiguousarray(
        xT.reshape(KD, 128, NS, 512).transpose(1, 2, 0, 3)).astype(E4NP)
    x16S = swz(xT[:, 0:128]).astype(bf)

    # causal boundary mask for diagonal tiles: mask[p, f] = 1 if f >= p
    f = np.arange(128)[None, :]
    p = np.arange(128)[:, None]
    masks = (f >= p).astype(bf)

    ones_col = np.ones((1, 128), np.float32)
    ones128 = np.ones((128, 16), bf)
    ones82 = np.ones((128, 2, 16), E4NP)

    wq_s = np.asarray(wq, np.float32) * w_ln[None, :] * WS
    wk_s = np.asarray(wk, np.float32) * w_ln[None, :] * WS
    wv_s = np.asarray(wv, np.float32) * w_ln[None, :] * WVS
    wo32 = np.asarray(wo, np.float32) / WVS

    in_maps = []
    for c in range(N_CORES):
        sl = slice(c * NL, (c + 1) * NL)
        woT = np.ascontiguousarray(wo32[:, sl].T)      # [NL, T]
        in_maps.append({
            "x8S": x8S,
            "x16S": x16S,
            "wq8S": swz(wq_s[sl].T).astype(E4NP),
            "wk8S": swz(wk_s[sl].T).astype(E4NP),
            "wv8S": swz(wv_s[sl].T).astype(E4NP),
            "wq16S": swz(wq_s[sl].T).astype(bf),
            "wk16S": swz(wk_s[sl].T).astype(bf),
            "wv16S": swz(wv_s[sl].T).astype(bf),
            "woS": swz(woT).astype(bf),
            "cosT": cosT,
            "sinT": sinT,
            "masks": masks,
            "ones_col": ones_col,
            "ones128": ones128,
            "ones82": ones82,
        })
    return in_maps


def _gather(x, results):
    acc = np.zeros((T, D), np.float32)
    for r in results:
        o = np.asarray(r["out"], np.float32)           # [128, TT, D]
        acc += o.transpose(1, 0, 2).reshape(T, D)
    return np.asarray(x, np.float32) + acc


def kernel(x, w_ln, wq, wk, wv, wo, cos, sin):
    nc = _build_program()
    in_maps = _host_prep(x, w_ln, wq, wk, wv, wo, cos, sin)
    t0 = time.time()
    res = run_bass_kernel_spmd(nc, in_maps, core_ids=list(range(N_CORES)))
    t1 = time.time()
    print(f"run_bass_kernel_spmd wall: {(t1 - t0) * 1e3:.1f} ms", file=sys.stderr)
    return _gather(x, res.results)
